# revision 3
# baseline (speedup 1.0000x reference)
"""Trainium2 Bass kernel for nn_AdaptiveDeblurBlock (dense CNN with a
pixel-adaptive 3x3 softmax-weighted reduce), SPMD across 8 NeuronCores.

Sharding: data-parallel over batch (4 images) x 2 spatial halves of H.
Core 2b+j handles image b, half j. Each core gets its 96-row half plus a
4-row halo (the full conv stack needs 4 rows of context), computes all
layers locally, and writes its 96 output rows. The bottom half is fed
VERTICALLY FLIPPED (with kh-flipped weights and tap-permuted w3) so that
one SPMD program serves both halves: the program always sees the image
boundary at its top and real halo rows at its bottom.

On-chip layout: every image buffer is a flat [P, 1 + nrows*194 + 1] fp16
tensor (lead zero, rows of [pad|192 valid|pad], tail zero), so each 3x3
conv is 9 flat-offset matmuls accumulated in PSUM, and the zero pads make
column-edge handling implicit. Softmax is computed as exp(l - ln(sum exp l))
using an extra K=1 matmul to spread -ln(s) over the 9 tap partitions; the
normalized kernel K goes to DRAM and is broadcast back to all 128
partitions with a 0-stride-partition DMA for the DVE tap-reduce.
"""
import os
import sys
from contextlib import ExitStack

import numpy as np

for _p in ("/opt/trn_rl_repo",):
    if _p not in sys.path and os.path.isdir(_p):
        sys.path.append(_p)

import concourse.bacc as bacc
import concourse.tile as tile
import concourse.mybir as mybir
from concourse.bass_utils import run_bass_kernel_spmd

DT16 = mybir.dt.float16
F32 = mybir.dt.float32
AF = mybir.ActivationFunctionType
NP16 = np.float16

B, C, H, W = 4, 128, 192, 192
HB = H // 2           # 96 output rows per core
RIN = HB + 4          # input rows per core (96 + 4-row halo)
RB = 8                # rows per adaptive-reduce block
Wp = W + 2

# tap permutation for vertically flipped cores: (di,dj) -> (2-di,dj)
_FLIP_PERM = np.array([3 * (2 - (t // 3)) + (t % 3) for t in range(9)])


# --------------------------------------------------------------------------
# device program
# --------------------------------------------------------------------------
def _build_kernel():
    nX = RIN + 1          # XB rows -1..RIN-1
    nC1 = RIN             # C1 rows -1..RIN-2
    nC2 = RIN - 2         # C2 rows 0..RIN-3
    nK = RIN - 2
    nA = RIN - 1          # A rows -1..RIN-3
    nF1 = RIN - 2         # F1 rows -1..RIN-4
    HOUT = RIN - 4

    def flatsz(nrows):
        return 1 + nrows * Wp + 1

    nc = bacc.Bacc()
    xb_ext = nc.declare_dram_parameter("xb", [C, flatsz(nX)], DT16, isOutput=False)
    w1_ext = nc.declare_dram_parameter("w1l", [128, 9 * 64], DT16, isOutput=False)
    w2_ext = nc.declare_dram_parameter("w2l", [64, 9 * 32], DT16, isOutput=False)
    w3_ext = nc.declare_dram_parameter("w3l", [32, 9], DT16, isOutput=False)
    f1_ext = nc.declare_dram_parameter("f1l", [128, 9 * 128], DT16, isOutput=False)
    f2_ext = nc.declare_dram_parameter("f2l", [128, 9 * 128], DT16, isOutput=False)
    on_ext = nc.declare_dram_parameter("ones", [16, 16], DT16, isOutput=False)
    bi_ext = nc.declare_dram_parameter("bias", [128, 8], F32, isOutput=False)
    out_ext = nc.declare_dram_parameter("out", [C, HOUT * W], F32, isOutput=True)

    kdram = nc.dram_tensor("kdram", [9, nK * Wp], DT16)

    def rview(t, off, nr, c0, c1):
        return t[:, off:off + nr * Wp].rearrange("p (r w) -> p r w", w=Wp)[:, :, c0:c1]

    with tile.TileContext(nc) as tc, ExitStack() as ctx:
        wpool = ctx.enter_context(tc.tile_pool(name="wpool", bufs=1))
        big = ctx.enter_context(tc.tile_pool(name="big", bufs=1))
        mid = ctx.enter_context(tc.tile_pool(name="mid", bufs=3))
        kbp = ctx.enter_context(tc.tile_pool(name="kbp", bufs=2))
        stg = ctx.enter_context(tc.tile_pool(name="stg", bufs=4))
        psum = ctx.enter_context(tc.tile_pool(name="psum", bufs=5, space="PSUM"))
        psum3 = ctx.enter_context(tc.tile_pool(name="psum3", bufs=2, space="PSUM"))
        psums = ctx.enter_context(tc.tile_pool(name="psums", bufs=1, space="PSUM"))

        W1 = wpool.tile([128, 9 * 64], DT16, tag="w1")
        W2 = wpool.tile([64, 9 * 32], DT16, tag="w2")
        W3 = wpool.tile([32, 9], DT16, tag="w3")
        F1W = wpool.tile([128, 9 * 128], DT16, tag="f1")
        F2W = wpool.tile([128, 9 * 128], DT16, tag="f2")
        ONES = wpool.tile([16, 16], DT16, tag="ones")
        BIAS = wpool.tile([128, 8], F32, tag="bias")
        nc.sync.dma_start(out=W1[:], in_=w1_ext[:])
        nc.sync.dma_start(out=W2[:], in_=w2_ext[:])
        nc.sync.dma_start(out=W3[:], in_=w3_ext[:])
        nc.sync.dma_start(out=F1W[:], in_=f1_ext[:])
        nc.sync.dma_start(out=F2W[:], in_=f2_ext[:])
        nc.sync.dma_start(out=ONES[:], in_=on_ext[:])
        nc.sync.dma_start(out=BIAS[:], in_=bi_ext[:])
        b1 = BIAS[0:64, 0:1]
        b2 = BIAS[0:32, 1:2]
        b3 = BIAS[0:9, 2:3]
        fb1 = BIAS[0:128, 3:4]
        fb2 = BIAS[0:128, 4:5]
        ones9 = ONES[0:9, 0:1]
        negones = ONES[0:1, 1:10]

        XB = big.tile([C, flatsz(nX)], DT16, tag="xb")
        nc.sync.dma_start(out=XB[:], in_=xb_ext[:])

        def pad_memset(buf, nrows, zero_top_row):
            if zero_top_row:
                nc.vector.memset(buf[:, 0:1 + Wp + 1], 0)
                first_pair_row = 1
            else:
                nc.vector.memset(buf[:, 0:2], 0)
                first_pair_row = 0
            npair = (nrows - 1) - first_pair_row
            if npair > 0:
                s = 1 + first_pair_row * Wp + (Wp - 1)
                nc.vector.memset(
                    buf[:, s:s + npair * Wp].rearrange(
                        "p (r w) -> p r w", w=Wp)[:, :, 0:2], 0)
            s = 1 + (nrows - 1) * Wp + (Wp - 1)
            nc.vector.memset(buf[:, s:s + 2], 0)

        C1 = big.tile([64, flatsz(nC1)], DT16, tag="slab1")
        C2 = big.tile([32, flatsz(nC2)], DT16, tag="slab2")
        pad_memset(C1, nC1, True)
        pad_memset(C2, nC2, False)

        def conv_block(r, nr, wtile, m, k, src, bias_ap, dst, func, rlo_dst=-1):
            N = nr * Wp
            pt = psum.tile([128, 512], F32, tag="mm")
            for t in range(9):
                di, dj = divmod(t, 3)
                off = 1 + (r + di) * Wp + (dj - 1)
                nc.tensor.matmul(pt[:m, :N], wtile[:k, t * m:(t + 1) * m],
                                 src[:k, off:off + N], start=(t == 0), stop=(t == 8))
            psrc = pt[:m, :N].rearrange("p (r w) -> p r w", w=Wp)[:, :, 1:1 + W]
            pdst = rview(dst, 1 + (r - rlo_dst) * Wp + 1, nr, 0, W)
            nc.scalar.activation(pdst, psrc, func, bias=bias_ap)

        rows_c1 = RIN - 1
        rows_c2 = RIN - 2
        blocks1 = [(r, min(2, rows_c1 - r)) for r in range(0, rows_c1, 2)]
        blocks2 = [(r, min(2, rows_c2 - r)) for r in range(0, rows_c2, 2)]
        for i in range(len(blocks1) + 1):
            if i < len(blocks1):
                r, nr = blocks1[i]
                conv_block(r, nr, W1, 64, 128, XB, b1, C1, AF.Relu)
            if 1 <= i and i - 1 < len(blocks2):
                r, nr = blocks2[i - 1]
                conv_block(r, nr, W2, 32, 64, C1, b2, C2, AF.Relu, rlo_dst=0)

        A = big.tile([C, flatsz(nA)], DT16, tag="slab1")
        pad_memset(A, nA, True)

        def c3_softmax_block(r, nr):
            N = nr * Wp
            co = 1 + r * Wp
            lg = psum3.tile([9, 512], F32, tag="c3")
            sp = psums.tile([1, 512], F32, tag="s")
            nc.tensor.matmul(lg[:, :N], W3[:, :], C2[:, co:co + N],
                             start=True, stop=True)
            E = mid.tile([9, 512], DT16, tag="E")
            nc.scalar.activation(E[:, :N], lg[:, :N], AF.Exp, bias=b3)
            nc.tensor.matmul(sp[:, :N], ones9, E[:, :N], start=True, stop=True)
            LNS = mid.tile([1, 512], DT16, tag="lns")
            nc.scalar.activation(LNS[:, :N], sp[:, :N], AF.Ln)
            nc.tensor.matmul(lg[:, :N], W3[:, :], C2[:, co:co + N],
                             start=True, stop=False)
            nc.tensor.matmul(lg[:, :N], negones, LNS[:, :N],
                             start=False, stop=True)
            KT = mid.tile([9, 512], DT16, tag="K")
            nc.scalar.activation(KT[:, :N], lg[:, :N], AF.Exp, bias=b3)
            nc.gpsimd.dma_start(out=kdram[0:9, r * Wp:r * Wp + N], in_=KT[:, :N])

        def reduce_block(r0, nr):
            F = nr * Wp
            KB = kbp.tile([C, 2 + 9 * RB * Wp], DT16, tag="kb")
            src = kdram[0:9, r0 * Wp:r0 * Wp + F].partition_broadcast(C)
            nc.gpsimd.dma_start(
                out=KB[:, 1:1 + 9 * F].rearrange("p (t f) -> p t f", f=F), in_=src)
            T = kbp.tile([C, RB * Wp], DT16, tag="tmp")
            av = rview(A, 1 + (r0 + 1) * Wp + 1, nr, 0, W)
            for t in range(9):
                di, dj = divmod(t, 3)
                xv = rview(XB, 1 + (r0 + di) * Wp + dj, nr, 0, W)
                kv = rview(KB, 1 + t * F + 1, nr, 0, W)
                if t == 0:
                    nc.vector.tensor_mul(av, xv, kv)
                else:
                    tv = rview(T, 0, nr, 0, W)
                    nc.vector.tensor_mul(tv, xv, kv)
                    nc.vector.tensor_add(av, av, tv)

        rows_k = RIN - 2
        sub_r = 0
        for r0 in range(0, rows_k, RB):
            nr = min(RB, rows_k - r0)
            while sub_r < r0 + nr:
                c3_softmax_block(sub_r, min(2, rows_k - sub_r))
                sub_r += 2
            reduce_block(r0, nr)

        F1 = big.tile([C, flatsz(nF1)], DT16, tag="slab2")
        pad_memset(F1, nF1, True)

        def f2_block(r, nr):
            N = nr * Wp
            pt = psum.tile([128, 512], F32, tag="mm")
            for t in range(9):
                di, dj = divmod(t, 3)
                off = 1 + (r + di) * Wp + (dj - 1)
                nc.tensor.matmul(pt[:, :N], F2W[:, t * 128:(t + 1) * 128],
                                 F1[:, off:off + N], start=(t == 0), stop=(t == 8))
            psrc = pt[:, :N].rearrange("p (r w) -> p r w", w=Wp)[:, :, 1:1 + W]
            ST = stg.tile([128, 2 * W], F32, tag="st")
            pdst = ST[:, :nr * W].rearrange("p (r w) -> p r w", w=W)
            nc.scalar.activation(pdst, psrc, AF.Identity, bias=fb2)
            nc.sync.dma_start(out=out_ext[:, r * W:(r + nr) * W], in_=ST[:, :nr * W])

        rows_f1 = RIN - 3
        rows_f2 = RIN - 4
        blocksf1 = [(r, min(2, rows_f1 - r)) for r in range(0, rows_f1, 2)]
        blocksf2 = [(r, min(2, rows_f2 - r)) for r in range(0, rows_f2, 2)]
        for i in range(len(blocksf1) + 1):
            if i < len(blocksf1):
                r, nr = blocksf1[i]
                conv_block(r, nr, F1W, 128, 128, A, fb1, F1, AF.Relu)
            if 1 <= i and i - 1 < len(blocksf2):
                f2_block(*blocksf2[i - 1])

    nc.compile()
    return nc


# --------------------------------------------------------------------------
# host-side packing
# --------------------------------------------------------------------------
def _flip_taps(w):
    return np.ascontiguousarray(w[:, :, ::-1, :])


def _pack_weights(w1, b1, w2, b2, w3, b3, f1, fb1, f2, fb2, flipped):
    if flipped:
        w1, w2, f1, f2 = _flip_taps(w1), _flip_taps(w2), _flip_taps(f1), _flip_taps(f2)
        w3 = w3[_FLIP_PERM]
        b3 = b3[_FLIP_PERM]
    d = {}
    d["w1l"] = np.ascontiguousarray(
        w1.transpose(1, 2, 3, 0).reshape(C, 9 * 64)).astype(NP16)
    d["w2l"] = np.ascontiguousarray(
        w2.transpose(1, 2, 3, 0).reshape(64, 9 * 32)).astype(NP16)
    d["w3l"] = np.ascontiguousarray(w3[:, :, 0, 0].T).astype(NP16)
    d["f1l"] = np.ascontiguousarray(
        f1.transpose(1, 2, 3, 0).reshape(C, 9 * C)).astype(NP16)
    d["f2l"] = np.ascontiguousarray(
        f2.transpose(1, 2, 3, 0).reshape(C, 9 * C)).astype(NP16)
    ones = np.zeros((16, 16), np.float32)
    ones[0:9, 0] = 1.0
    ones[0, 1:10] = -1.0
    d["ones"] = ones.astype(NP16)
    bias = np.zeros((C, 8), np.float32)
    bias[0:64, 0] = b1
    bias[0:32, 1] = b2
    # -12 logit shift keeps exp() within fp16 range; cancels exactly in softmax
    bias[0:9, 2] = b3 - 12.0
    bias[0:C, 3] = fb1
    bias[0:C, 4] = fb2
    d["bias"] = bias
    return d


def _pack_x(x_loc):
    nX = RIN + 1
    xb = np.zeros((C, 1 + nX * Wp + 1), np.float32)
    v = xb[:, 1:1 + nX * Wp].reshape(C, nX, Wp)
    v[:, 1:, 1:1 + W] = x_loc
    return xb.astype(NP16)


_NC_CACHE = {}


def _get_nc():
    if "nc" not in _NC_CACHE:
        _NC_CACHE["nc"] = _build_kernel()
    return _NC_CACHE["nc"]


def _make_in_maps(x, w1, b1, w2, b2, w3, b3, f1, fb1, f2, fb2):
    x = np.asarray(x, np.float32)
    args = [np.asarray(a, np.float32) for a in
            (w1, b1, w2, b2, w3, b3, f1, fb1, f2, fb2)]
    wtop = _pack_weights(*args, flipped=False)
    wbot = _pack_weights(*args, flipped=True)
    in_maps = []
    for b in range(B):
        for j in range(2):
            if j == 0:
                xloc = x[b, :, 0:RIN, :]
                wd = wtop
            else:
                xloc = np.ascontiguousarray(x[b, :, H - RIN:H, :][:, ::-1, :])
                wd = wbot
            m = dict(wd)
            m["xb"] = _pack_x(xloc)
            in_maps.append(m)
    return in_maps


def _assemble(results):
    out = np.zeros((B, C, H, W), np.float32)
    for b in range(B):
        out[b, :, 0:HB, :] = results[2 * b]["out"].reshape(C, HB, W)
        out[b, :, HB:H, :] = results[2 * b + 1]["out"].reshape(C, HB, W)[:, ::-1, :]
    return out


def kernel(x, w1, b1, w2, b2, w3, b3, f1, fb1, f2, fb2):
    nc = _get_nc()
    in_maps = _make_in_maps(x, w1, b1, w2, b2, w3, b3, f1, fb1, f2, fb2)
    res = run_bass_kernel_spmd(nc, in_maps, core_ids=list(range(8)))
    return _assemble(res.results)


# revision 5
# speedup vs baseline: 20.7925x; 20.7925x over previous
"""Trainium2 Bass kernel for nn_AdaptiveDeblurBlock, SPMD across 8 NeuronCores.

Host side shards batch x H-halves (4 images x 2 halves = 8 cores) with a
vertical-flip trick so one SPMD program serves both halves: every core sees
the image boundary at its top and real 4-row halo at its bottom. The device
program runs the full conv stack locally with the f1/f2 feature convs
interleaved into the softmax/reduce phase; see build_kernel for details.
"""
import os, sys
for _p in ("/opt/trn_rl_repo",):
    if _p not in sys.path and os.path.isdir(_p):
        sys.path.append(_p)

"""Bass/Tile kernel builder for the AdaptiveDeblurBlock, one NeuronCore SPMD program.

V2: wavefront emission — after the c1/c2 phase, each reduce-block iteration
emits its c3/softmax chain, the KB broadcast + DVE tap-reduce, and then all
newly-enabled f1/f2 blocks, so f1/f2 matmuls fill the PE stalls that the
serial c3->ACT->c3 softmax chain would otherwise cause. The XB input DMA is
chunked so conv1 starts as soon as the first rows land. A shifted copy of
the x window (xs) keeps all DVE tap-multiplies 4B-aligned (2x mode).
"""
from contextlib import ExitStack

import concourse.bass as bass
import concourse.bacc as bacc
import concourse.tile as tile
import concourse.mybir as mybir

DT16 = mybir.dt.float16
F32 = mybir.dt.float32
AF = mybir.ActivationFunctionType


def build_kernel(RIN=100, W=192, C=128, RB=6, compile_=True, with_tick=True):
    Wp = W + 2
    nX = RIN + 1          # XB rows -1..RIN-1
    nC1 = RIN             # C1 rows -1..RIN-2 (computed 0..RIN-2)
    nC2 = RIN - 2         # C2 rows 0..RIN-3
    nK = RIN - 2          # K rows 0..RIN-3
    nA = RIN - 1          # A rows -1..RIN-3
    nF1 = RIN - 2         # F1 rows -1..RIN-4 (computed 0..RIN-4)
    HOUT = RIN - 4

    def flatsz(nrows):
        return 1 + nrows * Wp + 1

    nc = bacc.Bacc()
    xb_ext = nc.declare_dram_parameter("xb", [C, flatsz(nX)], DT16, isOutput=False)
    w1_ext = nc.declare_dram_parameter("w1l", [128, 9 * 64], DT16, isOutput=False)
    w2_ext = nc.declare_dram_parameter("w2l", [64, 9 * 32], DT16, isOutput=False)
    w3_ext = nc.declare_dram_parameter("w3l", [32, 9], DT16, isOutput=False)
    f1_ext = nc.declare_dram_parameter("f1l", [128, 9 * 128], DT16, isOutput=False)
    f2_ext = nc.declare_dram_parameter("f2l", [128, 9 * 128], DT16, isOutput=False)
    on_ext = nc.declare_dram_parameter("ones", [16, 16], DT16, isOutput=False)
    bi_ext = nc.declare_dram_parameter("bias", [128, 8], F32, isOutput=False)
    tick_ext = tock_ext = None
    if with_tick:
        tick_ext = nc.declare_dram_parameter("tick", [1, 16], F32, isOutput=False)
    out_ext = nc.declare_dram_parameter("out", [C, HOUT * W], F32, isOutput=True)
    if with_tick:
        tock_ext = nc.declare_dram_parameter("tock", [1, 16], F32, isOutput=True)

    kdram = nc.dram_tensor("kdram", [9, nK * Wp], DT16)

    def rview(t, off, nr, c0, c1):
        return t[:, off:off + nr * Wp].rearrange("p (r w) -> p r w", w=Wp)[:, :, c0:c1]

    with tile.TileContext(nc) as tc, ExitStack() as ctx:
        wpool = ctx.enter_context(tc.tile_pool(name="wpool", bufs=1))
        big = ctx.enter_context(tc.tile_pool(name="big", bufs=1))
        mid = ctx.enter_context(tc.tile_pool(name="mid", bufs=3))
        kbp = ctx.enter_context(tc.tile_pool(name="kbp", bufs=2))
        stg = ctx.enter_context(tc.tile_pool(name="stg", bufs=4))
        psum = ctx.enter_context(tc.tile_pool(name="psum", bufs=3, space="PSUM"))
        psum3 = ctx.enter_context(tc.tile_pool(name="psum3", bufs=3, space="PSUM"))
        psums = ctx.enter_context(tc.tile_pool(name="psums", bufs=2, space="PSUM"))

        if with_tick:
            nc.sync.dma_start(out=tock_ext[:], in_=tick_ext[:])

        W1 = wpool.tile([128, 9 * 64], DT16, tag="w1")
        W2 = wpool.tile([64, 9 * 32], DT16, tag="w2")
        W3 = wpool.tile([32, 9], DT16, tag="w3")
        F1W = wpool.tile([128, 9 * 128], DT16, tag="f1")
        F2W = wpool.tile([128, 9 * 128], DT16, tag="f2")
        ONES = wpool.tile([16, 16], DT16, tag="ones")
        BIAS = wpool.tile([128, 8], F32, tag="bias")
        nc.sync.dma_start(out=W1[:], in_=w1_ext[:])
        nc.sync.dma_start(out=W2[:], in_=w2_ext[:])
        nc.sync.dma_start(out=W3[:], in_=w3_ext[:])
        nc.sync.dma_start(out=F1W[:], in_=f1_ext[:])
        nc.sync.dma_start(out=F2W[:], in_=f2_ext[:])
        nc.sync.dma_start(out=ONES[:], in_=on_ext[:])
        nc.sync.dma_start(out=BIAS[:], in_=bi_ext[:])
        b1 = BIAS[0:64, 0:1]
        b2 = BIAS[0:32, 1:2]
        b3 = BIAS[0:9, 2:3]
        fb1 = BIAS[0:128, 3:4]
        fb2 = BIAS[0:128, 4:5]
        ones9 = ONES[0:9, 0:1]
        negones = ONES[0:1, 1:10]

        # XB input, chunked so conv1's first blocks start early
        XB = big.tile([C, flatsz(nX)], DT16, tag="xb")
        CHUNK = 16 * Wp
        pos = 0
        while pos < flatsz(nX):
            end = min(pos + CHUNK, flatsz(nX))
            nc.sync.dma_start(out=XB[:, pos:end], in_=xb_ext[:, pos:end])
            pos = end

        def pad_memset(buf, nrows, zero_top_row):
            if zero_top_row:
                nc.vector.memset(buf[:, 0:1 + Wp + 1], 0)
                first_pair_row = 1
            else:
                nc.vector.memset(buf[:, 0:2], 0)
                first_pair_row = 0
            npair = (nrows - 1) - first_pair_row
            if npair > 0:
                s = 1 + first_pair_row * Wp + (Wp - 1)
                nc.vector.memset(
                    buf[:, s:s + npair * Wp].rearrange(
                        "p (r w) -> p r w", w=Wp)[:, :, 0:2], 0)
            s = 1 + (nrows - 1) * Wp + (Wp - 1)
            nc.vector.memset(buf[:, s:s + 2], 0)

        C1 = big.tile([64, flatsz(nC1)], DT16, tag="slab1")
        C2 = big.tile([32, flatsz(nC2)], DT16, tag="slab2")
        pad_memset(C1, nC1, True)
        pad_memset(C2, nC2, False)

        def conv_block(r, nr, wtile, m, k, src, bias_ap, dst, func, rlo_dst=-1):
            N = nr * Wp
            pt = psum.tile([128, 512], F32, tag="mm")
            for t in range(9):
                di, dj = divmod(t, 3)
                off = 1 + (r + di) * Wp + (dj - 1)
                nc.tensor.matmul(pt[:m, :N], wtile[:k, t * m:(t + 1) * m],
                                 src[:k, off:off + N], start=(t == 0), stop=(t == 8))
            psrc = pt[:m, :N].rearrange("p (r w) -> p r w", w=Wp)[:, :, 1:1 + W]
            pdst = rview(dst, 1 + (r - rlo_dst) * Wp + 1, nr, 0, W)
            nc.scalar.activation(pdst, psrc, func, bias=bias_ap)

        # ---- conv1 / conv2, interleaved emission ----
        rows_c1 = RIN - 1
        rows_c2 = RIN - 2
        blocks1 = [(r, min(2, rows_c1 - r)) for r in range(0, rows_c1, 2)]
        blocks2 = [(r, min(2, rows_c2 - r)) for r in range(0, rows_c2, 2)]
        for i in range(len(blocks1) + 1):
            if i < len(blocks1):
                r, nr = blocks1[i]
                conv_block(r, nr, W1, 64, 128, XB, b1, C1, AF.Relu)
            if 1 <= i and i - 1 < len(blocks2):
                r, nr = blocks2[i - 1]
                conv_block(r, nr, W2, 32, 64, C1, b2, C2, AF.Relu, rlo_dst=0)

        # ---- wavefront: c3/softmax + reduce + f1 + f2, merged ----
        # XB is dead after conv1 (the reduce re-reads x from DRAM), so A
        # takes its slab; F1 takes C1's. C2 stays live for the c3 blocks.
        A = big.tile([C, flatsz(nA)], DT16, tag="xb")
        pad_memset(A, nA, True)
        F1 = big.tile([C, flatsz(nF1)], DT16, tag="slab1")
        pad_memset(F1, nF1, True)

        def c3_stage1(r, nr):
            N = nr * Wp
            co = 1 + r * Wp
            lg = psum3.tile([9, 512], F32, tag="c3")
            nc.tensor.matmul(lg[:, :N], W3[:, :], C2[:, co:co + N],
                             start=True, stop=True)
            E = mid.tile([9, 512], DT16, tag="E")
            nc.scalar.activation(E[:, :N], lg[:, :N], AF.Exp, bias=b3)
            return E

        def c3_stage2(r, nr, E):
            N = nr * Wp
            sp = psums.tile([1, 512], F32, tag="s")
            nc.tensor.matmul(sp[:, :N], ones9, E[:, :N], start=True, stop=True)
            LNS = mid.tile([1, 512], DT16, tag="lns")
            nc.scalar.activation(LNS[:, :N], sp[:, :N], AF.Ln)
            return LNS

        def c3_stage3(r, nr, LNS):
            N = nr * Wp
            co = 1 + r * Wp
            lg2 = psum3.tile([9, 512], F32, tag="c3")
            nc.tensor.matmul(lg2[:, :N], W3[:, :], C2[:, co:co + N],
                             start=True, stop=False)
            nc.tensor.matmul(lg2[:, :N], negones, LNS[:, :N],
                             start=False, stop=True)
            KT = mid.tile([9, 512], DT16, tag="K")
            nc.scalar.activation(KT[:, :N], lg2[:, :N], AF.Exp, bias=b3)
            nc.gpsimd.dma_start(out=kdram[0:9, r * Wp:r * Wp + N], in_=KT[:, :N])

        def reduce_block(r0, nr):
            F = nr * Wp
            KB = kbp.tile([C, 2 + 9 * RB * Wp], DT16, tag="kb")
            src = kdram[0:9, r0 * Wp:r0 * Wp + F].partition_broadcast(C)
            nc.gpsimd.dma_start(
                out=KB[:, 1:1 + 9 * F].rearrange("p (t f) -> p t f", f=F), in_=src)
            # x windows re-fetched from DRAM at two different element bases so
            # that every tap view lands 4B-aligned (DVE 2x mode):
            #   xs0[f] = xdram[1 + r0*Wp + f]  -> taps dj in {0,2} at f = di*Wp+dj+c
            #   xs1[f] = xdram[r0*Wp + f]      -> taps dj == 1    at f = di*Wp+2+c
            xs0 = kbp.tile([C, (RB + 3) * Wp], DT16, tag="xs0")
            nc.sync.dma_start(
                out=xs0[:, 0:(nr + 2) * Wp],
                in_=xb_ext[:, 1 + r0 * Wp: 1 + r0 * Wp + (nr + 2) * Wp])
            xs1 = kbp.tile([C, (RB + 3) * Wp + 2], DT16, tag="xs1")
            nc.sync.dma_start(
                out=xs1[:, 0:(nr + 2) * Wp + 2],
                in_=xb_ext[:, r0 * Wp: r0 * Wp + (nr + 2) * Wp + 2])
            T = kbp.tile([C, RB * Wp], DT16, tag="tmp")
            av = rview(A, 1 + (r0 + 1) * Wp + 1, nr, 0, W)
            for t in range(9):
                di, dj = divmod(t, 3)
                if dj == 1:
                    xv = rview(xs1, di * Wp + 2, nr, 0, W)
                else:
                    xv = rview(xs0, di * Wp + dj, nr, 0, W)
                kv = rview(KB, 1 + t * F + 1, nr, 0, W)
                if t == 0:
                    nc.vector.tensor_mul(av, xv, kv)
                else:
                    tv = rview(T, 0, nr, 0, W)
                    nc.vector.tensor_mul(tv, xv, kv)
                    nc.vector.tensor_add(av, av, tv)

        def f2_block(r, nr):
            N = nr * Wp
            pt = psum.tile([128, 512], F32, tag="mm")
            for t in range(9):
                di, dj = divmod(t, 3)
                off = 1 + (r + di) * Wp + (dj - 1)
                nc.tensor.matmul(pt[:, :N], F2W[:, t * 128:(t + 1) * 128],
                                 F1[:, off:off + N], start=(t == 0), stop=(t == 8))
            psrc = pt[:, :N].rearrange("p (r w) -> p r w", w=Wp)[:, :, 1:1 + W]
            ST = stg.tile([128, 2 * W], F32, tag="st")
            pdst = ST[:, :nr * W].rearrange("p (r w) -> p r w", w=W)
            nc.scalar.activation(pdst, psrc, AF.Identity, bias=fb2)
            nc.sync.dma_start(out=out_ext[:, r * W:(r + nr) * W], in_=ST[:, :nr * W])

        rows_k = RIN - 2
        rows_f1 = RIN - 3
        rows_f2 = RIN - 4
        blocksf1 = [(r, min(2, rows_f1 - r)) for r in range(0, rows_f1, 2)]
        blocksf2 = [(r, min(2, rows_f2 - r)) for r in range(0, rows_f2, 2)]
        state = {"if1": 0, "if2": 0, "f1_rows": 0}

        def c3_softmax_block(r, nr):
            E = c3_stage1(r, nr)
            LNS = c3_stage2(r, nr, E)
            c3_stage3(r, nr, LNS)

        def emit_one_f(a_rows, final=False):
            if state["if2"] < len(blocksf2):
                r2, n2 = blocksf2[state["if2"]]
                if final or r2 + n2 <= state["f1_rows"] - 1:
                    f2_block(r2, n2)
                    state["if2"] += 1
                    return True
            if state["if1"] < len(blocksf1):
                r, nr = blocksf1[state["if1"]]
                if final or r + nr <= a_rows - 1:
                    conv_block(r, nr, F1W, 128, 128, A, fb1, F1, AF.Relu)
                    state["if1"] += 1
                    state["f1_rows"] = r + nr
                    return True
            return False

        sub_r = 0
        for r0 in range(0, rows_k, RB):
            nr = min(RB, rows_k - r0)
            while sub_r < r0 + nr:
                c3_softmax_block(sub_r, min(2, rows_k - sub_r))
                sub_r += 2
            reduce_block(r0, nr)
            emit_one_f(r0 + nr)
            emit_one_f(r0 + nr)
        while emit_one_f(rows_k):
            pass
        while state["if1"] < len(blocksf1) or state["if2"] < len(blocksf2):
            emit_one_f(rows_k, final=True)

    if compile_:
        nc.compile()
    return nc


# ==========================================================================
# host side
# ==========================================================================
import numpy as np
from concourse.bass_utils import run_bass_kernel_spmd

NP16 = np.float16
B, C, H, W = 4, 128, 192, 192
HB = H // 2
RIN = HB + 4
Wp = W + 2

_FLIP_PERM = np.array([3 * (2 - (t // 3)) + (t % 3) for t in range(9)])


def _flip_taps(w):
    return np.ascontiguousarray(w[:, :, ::-1, :])


def _pack_weights(w1, b1, w2, b2, w3, b3, f1, fb1, f2, fb2, flipped):
    if flipped:
        w1, w2, f1, f2 = _flip_taps(w1), _flip_taps(w2), _flip_taps(f1), _flip_taps(f2)
        w3 = w3[_FLIP_PERM]
        b3 = b3[_FLIP_PERM]
    d = {}
    d["w1l"] = np.ascontiguousarray(
        w1.transpose(1, 2, 3, 0).reshape(C, 9 * 64)).astype(NP16)
    d["w2l"] = np.ascontiguousarray(
        w2.transpose(1, 2, 3, 0).reshape(64, 9 * 32)).astype(NP16)
    d["w3l"] = np.ascontiguousarray(w3[:, :, 0, 0].T).astype(NP16)
    d["f1l"] = np.ascontiguousarray(
        f1.transpose(1, 2, 3, 0).reshape(C, 9 * C)).astype(NP16)
    d["f2l"] = np.ascontiguousarray(
        f2.transpose(1, 2, 3, 0).reshape(C, 9 * C)).astype(NP16)
    ones = np.zeros((16, 16), np.float32)
    ones[0:9, 0] = 1.0
    ones[0, 1:10] = -1.0
    d["ones"] = ones.astype(NP16)
    bias = np.zeros((C, 8), np.float32)
    bias[0:64, 0] = b1
    bias[0:32, 1] = b2
    # -12 logit shift keeps exp() within fp16 range; cancels exactly in softmax
    bias[0:9, 2] = b3 - 12.0
    bias[0:C, 3] = fb1
    bias[0:C, 4] = fb2
    d["bias"] = bias
    d["tick"] = np.zeros((1, 16), np.float32)
    return d


def _pack_x(x_loc):
    nX = RIN + 1
    xb = np.zeros((C, 1 + nX * Wp + 1), np.float32)
    v = xb[:, 1:1 + nX * Wp].reshape(C, nX, Wp)
    v[:, 1:, 1:1 + W] = x_loc
    return xb.astype(NP16)


_NC_CACHE = {}


def _get_nc():
    if "nc" not in _NC_CACHE:
        _NC_CACHE["nc"] = build_kernel()
    return _NC_CACHE["nc"]


def _make_in_maps(x, w1, b1, w2, b2, w3, b3, f1, fb1, f2, fb2):
    x = np.asarray(x, np.float32)
    args = [np.asarray(a, np.float32) for a in
            (w1, b1, w2, b2, w3, b3, f1, fb1, f2, fb2)]
    wtop = _pack_weights(*args, flipped=False)
    wbot = _pack_weights(*args, flipped=True)
    in_maps = []
    for b in range(B):
        for j in range(2):
            if j == 0:
                xloc = x[b, :, 0:RIN, :]
                wd = wtop
            else:
                xloc = np.ascontiguousarray(x[b, :, H - RIN:H, :][:, ::-1, :])
                wd = wbot
            m = dict(wd)
            m["xb"] = _pack_x(xloc)
            in_maps.append(m)
    return in_maps


def _assemble(results):
    out = np.zeros((B, C, H, W), np.float32)
    for b in range(B):
        out[b, :, 0:HB, :] = results[2 * b]["out"].reshape(C, HB, W)
        out[b, :, HB:H, :] = results[2 * b + 1]["out"].reshape(C, HB, W)[:, ::-1, :]
    return out


def kernel(x, w1, b1, w2, b2, w3, b3, f1, fb1, f2, fb2):
    nc = _get_nc()
    in_maps = _make_in_maps(x, w1, b1, w2, b2, w3, b3, f1, fb1, f2, fb2)
    res = run_bass_kernel_spmd(nc, in_maps, core_ids=list(range(8)))
    return _assemble(res.results)
